# revision 8
# baseline (speedup 1.0000x reference)
"""Trainium2 Bass kernel for nn_ExtraPositionPromptSABottleneck.

Data-parallel over batch B=8 across 8 NeuronCores; each core computes one
batch element's full bottleneck block:

  x1 = silu(bn1(cv1 @ x))            [C=256, N=4096]
  q/e folded host-side; vT materialized via matmul (fp8, x16)
  S^T[m,n] = k^T q + (rel-pos term)  computed in transposed layout (f32r)
  softmax:   exp -> bf16 es quad tiles; row sums via balanced bf16 DVE
             tree + one ones-matmul; 16/s via fast-approx reciprocal,
             broadcast to [128,NBLK] with a rank-1 matmul; the normalized
             attention weights are quantized to fp8e4 (values 16*attn).
  out:       fp8 DoubleRow matmuls (2 m-blocks per instruction at double
             rate): psum = (16 v) @ (16 attn)^T = 256*out, evacuated to
             bf16 as-is (the 1/256 is folded into cv2 host-side).
  y = silu(bn2(cv2 @ out)) + x       silu computed as (z/2)(1+tanh(z/2))
             so ACT stays on the exp/tanh table set.

k_b/e_b biases drop out (softmax shift invariance); v_b folds into b2
(attention rows sum to one).
"""

import numpy as np
import ml_dtypes

import concourse.bass as bass
import concourse.tile as tile
from concourse import bacc, mybir
from concourse.bass_utils import run_bass_kernel_spmd

f32 = mybir.dt.float32
f32r = mybir.dt.float32r
bf16 = mybir.dt.bfloat16
fp8 = mybir.dt.float8e4
AF = mybir.ActivationFunctionType
DR = mybir.MatmulPerfMode.DoubleRow
Alu = mybir.AluOpType

B, DIMS, SIZE = 8, 512, 64
C = DIMS // 2              # 256
N = SIZE * SIZE            # 4096
NBLK = 512                 # column block (one PSUM bank of fp32)
NNB = N // NBLK            # 8 n blocks
MB = N // 128              # 32 m blocks
NPAIR = MB // 2            # 16 m-block pairs
NQUAD = MB // 4            # 8 m-block quads
EPS = 1e-5

V_SCALE = 16.0             # v quantized to fp8 as 16*v
A_SCALE = 16.0             # attn quantized to fp8 as 16*attn
OUT_FOLD = V_SCALE * A_SCALE   # psum = 256*out; folded into cv2 host-side


def bcast4(ap):
    """[128, F] AP -> [128, 4, F] with a stride-0 broadcast middle dim."""
    return bass.AP(ap.tensor, ap.offset, [ap.ap[0], (0, 4), ap.ap[1]])


def build_nc():
    nc = bacc.Bacc("TRN2", target_bir_lowering=False, debug=False)

    x_d = nc.dram_tensor("x", [DIMS, N], f32r, kind="ExternalInput")
    cv1_d = nc.dram_tensor("cv1_lhsT", [DIMS, C], f32r, kind="ExternalInput")
    b1_d = nc.dram_tensor("b1", [128, 2], f32, kind="ExternalInput")
    qw_d = nc.dram_tensor("q_lhsT", [C, C], f32r, kind="ExternalInput")
    vw_d = nc.dram_tensor("v_rhs", [C, C], f32r, kind="ExternalInput")
    pm_d = nc.dram_tensor("pmat", [C, N], f32r, kind="ExternalInput")
    cv2_d = nc.dram_tensor("cv2_bf", [C, DIMS], bf16, kind="ExternalInput")
    b2_d = nc.dram_tensor("b2", [128, 4], f32, kind="ExternalInput")
    onesb_d = nc.dram_tensor("ones_bf", [128, 1], bf16, kind="ExternalInput")
    ones1_d = nc.dram_tensor("ones_row", [1, 128], f32r, kind="ExternalInput")
    y_d = nc.dram_tensor("y", [DIMS, N], f32, kind="ExternalOutput")

    with tile.TileContext(nc) as tc:
        with (
            tc.tile_pool(name="wp", bufs=1) as wp,
            tc.tile_pool(name="qp", bufs=1) as qp_pool,
            tc.tile_pool(name="vtp", bufs=1) as vt_pool,
            tc.tile_pool(name="bigbuf", bufs=2) as bigbuf,
            tc.tile_pool(name="xpanels", bufs=3) as xp_pool,
            tc.tile_pool(name="esp", bufs=10) as es_pool,
            tc.tile_pool(name="es8p", bufs=4) as es8_pool,
            tc.tile_pool(name="outp", bufs=3) as out_pool,
            tc.tile_pool(name="work", bufs=6) as work,
            tc.tile_pool(name="tree", bufs=2) as tree_pool,
            tc.tile_pool(name="small", bufs=2) as small,
            tc.tile_pool(name="ohp", bufs=2) as oh_pool,
        ):
            # ---- weights / constants ----
            cv1_t = wp.tile([128, 4, C], f32r, tag="cv1_t")
            nc.sync.dma_start(cv1_t[:], cv1_d.rearrange("(k p) m -> p k m", p=128))
            b1_t = wp.tile([128, 2], f32, tag="b1_t")
            nc.sync.dma_start(b1_t[:], b1_d[:])
            xp_pre = {}
            for nb0 in range(2):
                xt = xp_pool.tile([128, 4, NBLK], f32r, tag="x",
                                  name=f"xp_pre{nb0}")
                for kc in range(4):
                    nc.sync.dma_start(
                        xt[:, kc, :],
                        x_d[bass.ts(kc, 128), bass.ts(nb0, NBLK)])
                xp_pre[nb0] = xt
            qw_t = wp.tile([128, 2, C], f32r, tag="qw_t")
            nc.sync.dma_start(qw_t[:], qw_d.rearrange("(k p) m -> p k m", p=128))
            vw_t = wp.tile([128, 2, C], f32r, tag="vw_t")
            nc.sync.dma_start(vw_t[:], vw_d.rearrange("(k p) m -> p k m", p=128))
            onesb_t = wp.tile([128, 1], bf16, tag="onesb_t")
            nc.sync.dma_start(onesb_t[:], onesb_d[:])
            ones1_t = wp.tile([1, 128], f32r, tag="ones1_t")
            nc.sync.dma_start(ones1_t[:], ones1_d[:])
            b2_t = wp.tile([128, 4], f32, tag="b2_t")
            nc.sync.dma_start(b2_t[:], b2_d[:])
            cv2_t = wp.tile([128, 2, DIMS], bf16, tag="cv2_t")
            nc.sync.dma_start(cv2_t[:], cv2_d.rearrange("(k p) m -> p k m", p=128))

            # ---- persistent big tensors ----
            qp_t = [qp_pool.tile([128, N], f32r, tag=f"qp{c}", name=f"qp{c}")
                    for c in range(2)]
            # vt8[p, j, i, c] = 16*v[c, m=(2j+i)*128+p]
            vt_t = vt_pool.tile([128, NPAIR, 2, C], fp8, tag="vt")
            x1_t = [bigbuf.tile([128, N], f32r, tag="big", name=f"x1_{c}")
                    for c in range(2)]
            ke_t = x1_t  # cc^T = x1^T (kw^T qw) x1: x1 is the k-side operand

            # =========== Phase A+B: x -> x1 -> q,vT (per n-block) ======
            pm_tiles = {}

            def fetch_pm(nbf):
                ts_ = []
                for cb in range(2):
                    t = oh_pool.tile([128, NBLK], f32r, tag=f"pm{cb}",
                                     name=f"pm{cb}_{nbf}")
                    nc.sync.dma_start(
                        t[:], pm_d[bass.ts(cb, 128), bass.ts(nbf, NBLK)])
                    ts_.append(t)
                pm_tiles[nbf] = ts_
            fetch_pm(0)
            with tc.tile_pool(name="psAB", bufs=6, space="PSUM") as psAB:
                for nb in range(NNB):
                    ns = bass.ts(nb, NBLK)
                    if nb + 1 < NNB:
                        fetch_pm(nb + 1)
                    if nb in xp_pre:
                        xt = xp_pre[nb]
                    else:
                        xt = xp_pool.tile([128, 4, NBLK], f32r, tag="x",
                                          name=f"xp_{nb}")
                        nc.sync.dma_start(
                            xt[:],
                            x_d.rearrange("(k p) n -> p k n", p=128)[:, :, ns])
                    xp = [xt[:, kc, :] for kc in range(4)]
                    # x1 = silu(cv1' @ x + b1')
                    for cb in range(2):
                        ps = psAB.tile([128, NBLK], f32, tag="ps", name=f"x1ps{nb}_{cb}")
                        for kc in range(4):
                            nc.tensor.matmul(ps[:], cv1_t[:, kc, bass.ts(cb, 128)],
                                             xp[kc], start=(kc == 0), stop=(kc == 3))
                        nc.scalar.activation(x1_t[cb][:, ns], ps[:], AF.Silu,
                                             bias=b1_t[:, cb:cb + 1])
                    # q (q_b folded into pmat)
                    for cb in range(2):
                        ps = psAB.tile([128, NBLK], f32, tag="ps", name=f"qps{nb}_{cb}")
                        for kc in range(2):
                            nc.tensor.matmul(ps[:], qw_t[:, kc, bass.ts(cb, 128)],
                                             x1_t[kc][:, ns], start=(kc == 0),
                                             stop=(kc == 1))
                        nc.vector.tensor_add(qp_t[cb][:, ns], ps[:],
                                             pm_tiles[nb][cb][:])
                    # vT tiles (16*v, fp8): 4 m-blocks in this n-block
                    for sb in range(4):
                        m = nb * 4 + sb
                        msl = bass.ts(m, 128)  # columns of x1
                        ps = psAB.tile([128, C], f32, tag="ps", name=f"vps{m}")
                        nc.tensor.matmul(ps[:], x1_t[0][:, msl], vw_t[:, 0, :],
                                         start=True, stop=False)
                        nc.tensor.matmul(ps[:], x1_t[1][:, msl], vw_t[:, 1, :],
                                         start=False, stop=True)
                        nc.vector.tensor_copy(vt_t[:, m // 2, m % 2, :], ps[:])

            # ====== Phase C+D: attention (fp8 DoubleRow out), conv2 ======
            out_pairs = {}   # nb -> [128, 2, NBLK] bf16 (256*out), cv2 rhs
            x2_tiles = {}
            pending = []     # deferred closures from the previous n-block

            with (
                tc.tile_pool(name="ps_st", bufs=2, space="PSUM") as ps_st,
                tc.tile_pool(name="ps_o", bufs=2, space="PSUM") as ps_o,
                tc.tile_pool(name="ps_m", bufs=2, space="PSUM") as ps_m,
            ):
                def make_nb_closures(nb, es_tiles, rcb_t):
                    ns = bass.ts(nb, NBLK)
                    ops = [None, None]
                    cls = []

                    def mk_quad(q):
                        def f():
                            es8 = es8_pool.tile([128, 4, NBLK], fp8, tag="es8",
                                                name=f"es8_{nb}_{q}")
                            nc.vector.tensor_mul(es8[:], es_tiles[q][:],
                                                 bcast4(rcb_t[:]))
                            if q == 0:
                                ops[0] = ps_o.tile([128, NBLK], f32, tag="oacc",
                                                   name=f"oacc{nb}_0")
                                ops[1] = ps_o.tile([128, NBLK], f32, tag="oacc",
                                                   name=f"oacc{nb}_1")
                            for h in range(2):
                                for cb in range(2):
                                    nc.tensor.matmul(
                                        ops[cb][:],
                                        vt_t[:, 2 * q + h, :, bass.ts(cb, 128)],
                                        es8[:, 2 * h:2 * h + 2, :],
                                        start=(q == 0 and h == 0),
                                        stop=(q == NQUAD - 1 and h == 1),
                                        perf_mode=DR)
                        return f

                    for q in range(NQUAD):
                        cls.append(mk_quad(q))

                    def evac():
                        op = out_pool.tile([128, 2, NBLK], bf16, tag="outp",
                                           name=f"outp{nb}")
                        for cb in range(2):
                            nc.vector.tensor_copy(op[:, cb, :], ops[cb][:])
                        out_pairs[nb] = op
                    cls.append(evac)

                    def mk_yd(ob):
                        def f():
                            u = ps_m.tile([128, NBLK], f32, tag="m",
                                          name=f"u{nb}_{ob}")
                            for kc in range(2):
                                nc.tensor.matmul(
                                    u[:], cv2_t[:, kc, bass.ts(ob, 128)],
                                    out_pairs[nb][:, kc, :],
                                    start=(kc == 0), stop=(kc == 1))
                            th = work.tile([128, NBLK], f32, tag="wk",
                                           name=f"th{nb}_{ob}")
                            nc.scalar.activation(th[:], u[:], AF.Tanh,
                                                 bias=b2_t[:, ob:ob + 1])
                            w2 = work.tile([128, NBLK], f32, tag="wk",
                                           name=f"w2_{nb}_{ob}")
                            nc.scalar.activation(w2[:], u[:], AF.Identity,
                                                 bias=b2_t[:, ob:ob + 1])
                            # res = (th + 1)*w2 + x2
                            m_ = work.tile([128, NBLK], f32, tag="wk",
                                           name=f"m{nb}_{ob}")
                            nc.vector.scalar_tensor_tensor(
                                m_[:], th[:], 1.0, w2[:],
                                Alu.add, Alu.mult)
                            res = work.tile([128, NBLK], f32, tag="wk",
                                            name=f"res{nb}_{ob}")
                            nc.gpsimd.tensor_add(res[:], m_[:],
                                                 x2_tiles[nb][:, ob, :])
                            nc.sync.dma_start(y_d[bass.ts(ob, 128), ns], res[:])
                        return f

                    for ob in range(4):
                        cls.append(mk_yd(ob))
                    return cls

                for nb in range(NNB):
                    ns = bass.ts(nb, NBLK)
                    # x2 residual prefetch for this block's conv2
                    x2t = xp_pool.tile([128, 4, NBLK], f32, tag="x",
                                       name=f"x2_{nb}")
                    nc.sync.dma_start(
                        x2t[:],
                        x_d.rearrange("(k p) n -> p k n", p=128)[:, :, ns]
                        .bitcast(f32))
                    x2_tiles[nb] = x2t

                    es_tiles = []
                    acc = tree_pool.tile([128, 4, NBLK], bf16, tag="tr",
                                         name=f"acc{nb}")
                    n_drain = len(pending)
                    drained = 0
                    for q in range(NQUAD):
                        es = es_pool.tile([128, 4, NBLK], bf16, tag="es",
                                          name=f"es{nb}_{q}")
                        for half in range(2):
                            st = ps_st.tile([128, 2, NBLK], f32, tag="st",
                                            name=f"st{nb}_{q}_{half}")
                            for hh in range(2):
                                mb = 4 * q + 2 * half + hh
                                for kc in range(2):
                                    nc.tensor.matmul(
                                        st[:, hh, :],
                                        ke_t[kc][:, bass.ts(mb, 128)],
                                        qp_t[kc][:, ns],
                                        start=(kc == 0), stop=(kc == 1))
                            # drain deferred work from the previous n-block
                            want = (n_drain * (2 * q + half + 1)) // NPAIR
                            while drained < want:
                                pending.pop(0)()
                                drained += 1
                            nc.scalar.activation(
                                es[:, 2 * half:2 * half + 2, :], st[:], AF.Exp)
                        if q == 0:
                            nc.vector.tensor_copy(acc[:], es[:])
                        else:
                            nc.vector.tensor_add(acc[:], acc[:], es[:])
                        es_tiles.append(es)
                    while pending:
                        pending.pop(0)()

                    # bf16 quad accumulator + in-place fold
                    esum = small.tile([128, NBLK], bf16, tag="sm",
                                      name=f"esum{nb}")
                    nc.vector.tensor_add(esum[:], acc[:, 0, :], acc[:, 1, :])
                    nc.vector.tensor_add(esum[:], esum[:], acc[:, 2, :])
                    nc.vector.tensor_add(esum[:], esum[:], acc[:, 3, :])

                    # rowsum -> fast reciprocal -> broadcast (16/s as bf16)
                    s_ps = ps_m.tile([1, NBLK], f32, tag="m", name=f"s{nb}")
                    nc.tensor.matmul(s_ps[:], onesb_t[:], esum[:],
                                     start=True, stop=True)
                    rc = small.tile([1, NBLK], f32, tag="sm1", name=f"rc{nb}")
                    nc.vector.reciprocal_approx_fast(rc[:], s_ps[:])
                    rcr = small.tile([1, NBLK], f32r, tag="smr", name=f"rcr{nb}")
                    nc.vector.tensor_copy(rcr[:], rc[:])
                    rps = ps_m.tile([128, NBLK], f32, tag="m", name=f"rb{nb}")
                    nc.tensor.matmul(rps[:], ones1_t[:], rcr[:],
                                     start=True, stop=True)
                    rcb = small.tile([128, NBLK], bf16, tag="smb",
                                     name=f"rcb{nb}")
                    nc.scalar.activation(rcb[:], rps[:], AF.Identity,
                                         scale=A_SCALE)

                    pending = make_nb_closures(nb, es_tiles, rcb)
                while pending:
                    pending.pop(0)()

    nc.compile()
    return nc


def prep_inputs(inputs):
    """Host-side folding of BN + weight layouts. Returns the shared in_map."""
    i = {k: np.asarray(v, dtype=np.float32) if np.asarray(v).dtype == np.float32
         else np.asarray(v) for k, v in inputs.items()}
    s1 = i["bn1_g"] / np.sqrt(i["bn1_v"] + EPS)
    cv1w = i["cv1_w"] * s1[:, None]                       # [C, DIMS]
    b1 = i["bn1_b"] - i["bn1_m"] * s1                     # [C]
    s2 = i["bn2_g"] / np.sqrt(i["bn2_v"] + EPS)
    cv2w = 0.5 * i["cv2_w"] * s2[:, None]                 # [DIMS, C] (1/2 for tanh-silu)
    # v_b folds into b2: attention rows sum to 1, so out += v_b per channel
    b2 = 0.5 * (i["bn2_b"] - i["bn2_m"] * s2) + cv2w @ i["v_b"]   # [DIMS]

    pos = (i["rel_h"] + i["rel_w"]).reshape(C, N).astype(np.float64)
    pmat = (i["e_w"].astype(np.float64).T @ pos
            + (i["k_w"].astype(np.float64).T @ i["q_b"].astype(np.float64))[:, None]
            ).astype(np.float32)

    # psum of the out matmul holds 256*out; fold 1/256 into cv2
    cv2_bf = (cv2w.T / OUT_FOLD).astype(ml_dtypes.bfloat16)   # [C, DIMS]

    return {
        "cv1_lhsT": np.ascontiguousarray(cv1w.T),         # [DIMS, C]
        "b1": np.ascontiguousarray(b1.reshape(2, 128).T),
        "q_lhsT": np.ascontiguousarray(
            (i["q_w"].astype(np.float64).T
             @ i["k_w"].astype(np.float64)).astype(np.float32)),
        "v_rhs": np.ascontiguousarray(V_SCALE * i["v_w"].T),  # [C, C]
        "pmat": pmat,
        "cv2_bf": np.ascontiguousarray(cv2_bf),
        "b2": np.ascontiguousarray(b2.reshape(4, 128).T),
        "ones_bf": np.ones((128, 1), ml_dtypes.bfloat16),
        "ones_row": np.ones((1, 128), np.float32),
    }


_NC = None


def run(inputs, trace=False):
    global _NC
    if _NC is None:
        _NC = build_nc()
    shared = prep_inputs(inputs)
    x = np.asarray(inputs["x"], dtype=np.float32)  # [B, DIMS, SIZE, SIZE]
    in_maps = []
    for b in range(B):
        m = dict(shared)
        m["x"] = np.ascontiguousarray(x[b].reshape(DIMS, N))
        in_maps.append(m)
    res = run_bass_kernel_spmd(_NC, in_maps, list(range(B)), trace=trace)
    out = np.stack([res.results[b]["y"].reshape(DIMS, SIZE, SIZE)
                    for b in range(B)], axis=0)
    return out.astype(np.float32), res.exec_time_ns


def kernel(**inputs) -> np.ndarray:
    out, _ = run(inputs, trace=False)
    return out
